# revision 26
# baseline (speedup 1.0000x reference)
"""Trainium2 Bass kernel for nn_BSAM_60129542251.

Conv-QKV self-attention block (B=4, C=64, H=W=64):
  Q = conv3x3(A1_B, w1)  -> [b, 32, 4096]
  K = conv3x3(A1_C, w2)  -> [b, 32, 4096]
  V = conv3x3(A1_C, w3)  -> [b, 64, 4096]
  E = softmax(Q^T K) V^T -> [b, 4096, 64];  out = E^T + A1_C

Sharding: 8 cores; core i handles sample b=i//2, row-half i%2 (2048 query
rows). K/V convs are duplicated within a sample pair; Q conv runs on the
core's half only. Attention fully fused on-chip.

v2 structure (vs baseline): conv is interleaved with the first attention
m-tile so the ScalarE exp stream (the critical engine) starts ~6us in; all
bias+copy moves run on DVE, edge-correction adds on GpSimd, exp is the only
ScalarE work. Q conv emits plain [32, n] tiles (no 4x partition-group
replication) and ST matmuls take k_sb slices directly as stationary (no k4
swizzle). Transposes use a bf16 identity; corr matmuls use bf16 edge
columns as the moving operand. Inputs stream in column chunks; the PE is
pre-warmed with dummy matmuls so conv starts at full clock.

Attention per 512-row m-tile: 8 key groups of 4x128 keys; per group 4
ST matmuls (K chunk stationary [32,128], Q moving [32,512]) into two
[128,1024] PSUM tiles, 2 exps, then E'^T += V'_k^T P with V' = [V | ones]
so row 64 accumulates softmax denominators. Normalize via reciprocal +
gpsimd partition broadcast + mul + residual add (residual slice DMA'd from
the A1_C flat tensor off the critical path).

Conv = 9 shifted matmuls over a flat zero-padded input (rows 64:128 hold
the one-h-row-ahead copy so dy0/dy1 pack into one 128-contraction matmul);
row-wrap reads at w=0/63 are cancelled by edge-correction matmuls.
"""

import numpy as np

import concourse.bacc as bacc
import concourse.mybir as mybir
import concourse.tile as tile
from concourse import bass_utils
from concourse.masks import make_identity

F32 = mybir.dt.float32
F32R = mybir.dt.float32r
BF16 = mybir.dt.bfloat16
AF = mybir.ActivationFunctionType

B, C, CH, H, W = 4, 64, 32, 64, 64
N = H * W                     # 4096 keys
M = N // 2                    # 2048 query rows per core
NCORES = 8
XC_LEN = 4352                 # padded flat A1_C: 66*64+2 = 4226, padded up
XB_LEN = 2304                 # padded flat A1_B half: 34*64+2 = 2178, padded up
NKC = N // 128                # 32 key chunks
NKG = NKC // 4                # 8 key groups
MTA = 512                     # attention m-tile
NWARM = 24                    # PE p-state warmup matmuls

_cache = {}


def _r32r(x):
    """Round fp32 -> float32r (zero low 12 mantissa bits, round to nearest)."""
    x = np.ascontiguousarray(x, np.float32)
    b = x.view(np.uint32).astype(np.uint64)
    out = (((b + np.uint64(1 << 11)) & np.uint64(0xFFFFF000)).astype(np.uint32)).view(np.float32)
    return np.ascontiguousarray(out)


def _build(dbg=False):
    nc = bacc.Bacc("TRN2", target_bir_lowering=False, debug=False)

    xc = nc.dram_tensor("xc", [128, XC_LEN], F32R, kind="ExternalInput")
    xb = nc.dram_tensor("xb", [128, XB_LEN], F32R, kind="ExternalInput")
    w23t = nc.dram_tensor("w23t", [128, 6 * 128], F32R, kind="ExternalInput")
    w1c = nc.dram_tensor("w1c", [128, 6 * 32], F32R, kind="ExternalInput")
    wcme = nc.dram_tensor("wcme", [C, 970], F32R, kind="ExternalInput")
    bia = nc.dram_tensor("bia", [128, 2], F32, kind="ExternalInput")
    resid = nc.dram_tensor("resid", [C, M], F32, kind="ExternalInput")
    out_d = nc.dram_tensor("out", [C, M], F32, kind="ExternalOutput")
    if dbg:
        k_d = nc.dram_tensor("k_dbg", [CH, N], F32, kind="ExternalOutput")
        q_d = nc.dram_tensor("q_dbg", [CH, M], F32, kind="ExternalOutput")
        v_d = nc.dram_tensor("v_dbg", [128, NKC * 65], BF16, kind="ExternalOutput")
        c_d = nc.dram_tensor("c_dbg", [128, 320], F32, kind="ExternalOutput")

    with tile.TileContext(nc) as tc:
        with (
            tc.tile_pool(name="big", bufs=1) as big,
            tc.tile_pool(name="work", bufs=2) as work,
            tc.tile_pool(name="expool", bufs=22) as expool,
        ):
            xc_sb = big.tile([128, XC_LEN], F32R, tag="xc")
            xb_sb = big.tile([128, XB_LEN], F32R, tag="xb")
            w23_sb = big.tile([128, 6 * 128], F32R, tag="w23")
            w1c_sb = big.tile([128, 6 * 32], F32R, tag="w1")
            wc1_sb = big.tile([C, 192], F32R, tag="wc1")
            wcK_sb = big.tile([C, 192], F32R, tag="wcK")
            wcV_sb = big.tile([C, 384], F32R, tag="wcV")
            me_sb = big.tile([C, 202], F32R, tag="me")
            bia_sb = big.tile([128, 2], F32, tag="bia")
            res_sb = big.tile([C, M], F32, tag="res")
            corrS = big.tile([128, 320], F32R, tag="corrS")
            k_sb = big.tile([CH, N], F32R, tag="k")
            qt_sb = big.tile([CH, M], F32R, tag="qt")
            v_sb = big.tile([128, NKC * 65], BF16, tag="v")
            ident = big.tile([C, C], F32R, tag="ident")
            warm_in = big.tile([128, 256], F32R, tag="warm_in")

            # ---- warmup / constants (first so PE/DVE start immediately) ----
            nc.vector.memset(warm_in[:].bitcast(F32), 0.0)
            identf = big.tile([C, C], F32, tag="identf")
            make_identity(nc, identf[:])
            nc.vector.tensor_copy(ident[:], identf[:])
            ones_f32 = big.tile([128, NKC], F32, tag="ones")
            nc.vector.memset(ones_f32[:], 1.0)
            warm = big.tile([128, 1], F32, tag="warm")
            nc.scalar.activation(warm[:], ones_f32[:, 0:1], AF.Exp)
            v3 = v_sb[:].rearrange("p (a b) -> p a b", b=65)
            nc.vector.tensor_copy(v3[:, :, 64], ones_f32[:])

            # ---- input DMAs, ordered for earliest first conv tile ----
            # The DMA device is FIFO across queues, so criticals go first:
            # weights + first x chunks, then bulk; resid is emitted late in
            # the SP stream (needed only at mt0 normalize).
            XCC = [0, 1216, 2752, 4352]
            XBC = [0, 704, 2304]
            nc.sync.dma_start(out=w23_sb[:], in_=w23t.ap())
            nc.scalar.dma_start(out=w1c_sb[:], in_=w1c.ap())
            nc.sync.dma_start(out=xb_sb[:, XBC[0]:XBC[1]], in_=xb.ap()[:, XBC[0]:XBC[1]])
            nc.scalar.dma_start(out=wc1_sb[:], in_=wcme.ap()[:, 0:192])
            nc.scalar.dma_start(out=wcK_sb[:], in_=wcme.ap()[:, 192:384])
            nc.scalar.dma_start(out=wcV_sb[:], in_=wcme.ap()[:, 384:768])
            nc.scalar.dma_start(out=me_sb[:], in_=wcme.ap()[:, 768:970])
            nc.sync.dma_start(out=xc_sb[:, XCC[0]:XCC[1]], in_=xc.ap()[:, XCC[0]:XCC[1]])
            nc.scalar.dma_start(out=bia_sb[:], in_=bia.ap())
            nc.sync.dma_start(out=xc_sb[:, XCC[1]:XCC[2]], in_=xc.ap()[:, XCC[1]:XCC[2]])
            nc.scalar.dma_start(out=xb_sb[:, XBC[1]:XBC[2]], in_=xb.ap()[:, XBC[1]:XBC[2]])
            nc.sync.dma_start(out=xc_sb[:, XCC[2]:XCC[3]], in_=xc.ap()[:, XCC[2]:XCC[3]])

            ecl_sb = me_sb[:, 0:66]
            ecr_sb = me_sb[:, 66:133]
            ebl_sb = me_sb[:, 133:167]
            ebr_sb = me_sb[:, 167:202]

            with tc.tile_pool(name="aps", bufs=2, space="PSUM") as aps:
              def st_exp(mt, kg):
                  sts = []
                  for h in range(2):
                      st_h = aps.tile([128, 1024], F32, tag="st")
                      sts.append(st_h)
                  for i in range(4):
                      kk = kg * 4 + i
                      nc.tensor.matmul(
                          sts[i // 2][:, (i % 2) * 512:(i % 2 + 1) * 512],
                          k_sb[:, kk * 128:(kk + 1) * 128],
                          qt_sb[:, mt * MTA:(mt + 1) * MTA],
                          start=True, stop=True)
                  exs = []
                  for h in range(2):
                      ex = expool.tile([128, 1024], BF16, tag="ex")
                      nc.scalar.activation(ex[:], sts[h][:], AF.Exp)
                      exs.append(ex)
                  return exs

              def pv_group(et, kg, exs):
                  for i in range(4):
                      kk = kg * 4 + i
                      nc.tensor.matmul(
                          et[:],
                          v_sb[:, kk * 65:kk * 65 + 65],
                          exs[i // 2][:, (i % 2) * 512:(i % 2 + 1) * 512],
                          start=(kk == 0), stop=(kk == NKC - 1))

              def normalize(mt, et):
                  recip = work.tile([1, MTA], F32, tag="recip")
                  nc.vector.reciprocal(recip[:], et[64:65, :])
                  bc = work.tile([C, MTA], F32, tag="bc")
                  nc.gpsimd.partition_broadcast(bc[:], recip[:])
                  ot = work.tile([C, MTA], F32, tag="ot")
                  nc.vector.tensor_mul(ot[:], et[0:C, :], bc[:])
                  nc.vector.tensor_add(ot[:], ot[:], res_sb[:, mt * MTA:(mt + 1) * MTA])
                  nc.sync.dma_start(out=out_d.ap()[:, mt * MTA:(mt + 1) * MTA], in_=ot[:])

              with (
                tc.tile_pool(name="cps", bufs=2, space="PSUM") as cps,
                tc.tile_pool(name="vtp", bufs=1, space="PSUM") as vtp,
                tc.tile_pool(name="crp", bufs=1, space="PSUM") as crp,
              ):
                # PE p-state warmup: dummy matmuls bridge until conv inputs
                # arrive so real matmuls run at full clock.
                for i in range(NWARM):
                    wps = cps.tile([128, 256], F32, tag="kv")
                    nc.tensor.matmul(
                        wps[:], warm_in[:, 0:128], warm_in[:],
                        start=True, stop=True)

                def conv_mms(pk, x_sb, w_sb, j, co, rows):
                    for dx in range(3):
                        base = j * 512 + dx
                        nc.tensor.matmul(
                            pk[0:rows, :], w_sb[:, dx * co:(dx + 1) * co],
                            x_sb[:, base:base + 512],
                            start=(dx == 0), stop=False)
                    for dx in range(3):
                        base = j * 512 + 2 * 64 + dx
                        nc.tensor.matmul(
                            pk[0:rows, :], w_sb[0:C, (3 + dx) * co:(4 + dx) * co],
                            x_sb[0:C, base:base + 512],
                            start=False, stop=(dx == 2))

                corrB_L = corrS[0:CH, 0:32]    # Q left, per h-row of half
                corrB_R = corrS[0:CH, 32:64]
                corrC_L = corrS[:, 64:128]     # K left, per h-row
                corrC_R = corrS[:, 128:192]
                corrV_L = corrS[0:C, 192:256]  # V, on partitions 0:64
                corrV_R = corrS[0:C, 256:320]

                def kv_tile(j):
                    pk = cps.tile([128, 512], F32, tag="kv")
                    conv_mms(pk, xc_sb, w23_sb, j, 128, 128)
                    # bias-copy to SBUF, then edge corrections on the SBUF
                    # tiles (all f32r, partition base 0) on DVE
                    ksl = k_sb[:, j * 512:(j + 1) * 512]
                    nc.vector.tensor_scalar_add(ksl, pk[0:CH, :], bia_sb[0:CH, 0:1])
                    k3 = ksl.rearrange("p (h w) -> p h w", w=64)
                    nc.vector.tensor_add(k3[:, :, 0], k3[:, :, 0], corrC_L[0:CH, j * 8:(j + 1) * 8])
                    nc.vector.tensor_add(k3[:, :, 63], k3[:, :, 63], corrC_R[0:CH, j * 8:(j + 1) * 8])
                    vtmp = work.tile([C, 512], F32R, tag="vtmp")
                    nc.vector.tensor_scalar_add(vtmp[:], pk[64:128, :], bia_sb[64:128, 0:1])
                    v3t = vtmp[:].rearrange("p (h w) -> p h w", w=64)
                    nc.vector.tensor_add(v3t[:, :, 0], v3t[:, :, 0], corrV_L[:, j * 8:(j + 1) * 8])
                    nc.vector.tensor_add(v3t[:, :, 63], v3t[:, :, 63], corrV_R[:, j * 8:(j + 1) * 8])
                    for c4 in range(4):
                        kk = j * 4 + c4
                        tp = vtp.tile([128, C], F32R, tag="vt")
                        nc.tensor.transpose(tp[:], vtmp[:, c4 * 128:(c4 + 1) * 128], ident[:])
                        nc.vector.tensor_copy(v_sb[:, kk * 65:kk * 65 + 64], tp[:])

                def q_tile(j):
                    pq = cps.tile([128, 512], F32, tag="kv")
                    conv_mms(pq, xb_sb, w1c_sb, j, 32, CH)
                    qsl = qt_sb[:, j * 512:(j + 1) * 512]
                    nc.vector.tensor_scalar_add(qsl, pq[0:CH, :], bia_sb[0:CH, 1:2])
                    q3 = qsl.rearrange("p (h w) -> p h w", w=64)
                    nc.vector.tensor_add(q3[:, :, 0], q3[:, :, 0], corrB_L[:, j * 8:(j + 1) * 8])
                    nc.vector.tensor_add(q3[:, :, 63], q3[:, :, 63], corrB_R[:, j * 8:(j + 1) * 8])

                # conv for tile 0 + Q0 first, then corr matmuls (their edge
                # inputs land mid-conv; emitting them later avoids
                # head-of-line blocking the PE sequencer), then the mt0
                # ST/exp pipeline with conv prefetched 2 tiles ahead.
                # mt0's PVs are deferred into mt1/mt2 (mt0 is PE-bound).
                corrT = crp.tile([128, 320], F32, tag="corr")
                # V corrections (widest output-partition extent) must come
                # first in the accumulation group; later members must not
                # exceed the first's partition extent
                for dy in range(3):
                    nc.tensor.matmul(
                        corrT[0:C, 192:256],
                        wcV_sb[:, (2 * dy) * 64:(2 * dy + 1) * 64].bitcast(F32),
                        ecl_sb[:, dy:dy + 64].bitcast(F32),
                        start=(dy == 0), stop=False)
                    nc.tensor.matmul(
                        corrT[0:C, 256:320],
                        wcV_sb[:, (2 * dy + 1) * 64:(2 * dy + 2) * 64].bitcast(F32),
                        ecr_sb[:, dy + 1:dy + 65].bitcast(F32),
                        start=False, stop=False)
                for dy in range(3):
                    nc.tensor.matmul(
                        corrT[0:CH, 0:32],
                        wc1_sb[:, (2 * dy) * 32:(2 * dy + 1) * 32].bitcast(F32),
                        ebl_sb[:, dy:dy + 32].bitcast(F32),
                        start=False, stop=False)
                    nc.tensor.matmul(
                        corrT[0:CH, 32:64],
                        wc1_sb[:, (2 * dy + 1) * 32:(2 * dy + 2) * 32].bitcast(F32),
                        ebr_sb[:, dy + 1:dy + 33].bitcast(F32),
                        start=False, stop=False)
                for dy in range(3):
                    nc.tensor.matmul(
                        corrT[0:CH, 64:128],
                        wcK_sb[:, (2 * dy) * 32:(2 * dy + 1) * 32].bitcast(F32),
                        ecl_sb[:, dy:dy + 64].bitcast(F32),
                        start=False, stop=False)
                    nc.tensor.matmul(
                        corrT[0:CH, 128:192],
                        wcK_sb[:, (2 * dy + 1) * 32:(2 * dy + 2) * 32].bitcast(F32),
                        ecr_sb[:, dy + 1:dy + 65].bitcast(F32),
                        start=False, stop=(dy == 2))
                nc.vector.tensor_copy(corrS[:], corrT[:])

                kv_tile(0)
                q_tile(0)
                kv_tile(1)
                exs0 = []
                for j in range(NKG):
                    if j + 2 < NKG:
                        kv_tile(j + 2)
                    exs0.append(st_exp(0, j))
                for j in range(1, 4):
                    q_tile(j)
                if dbg:
                    nc.sync.dma_start(out=k_d.ap(), in_=k_sb[:].bitcast(F32))
                    nc.sync.dma_start(out=q_d.ap(), in_=qt_sb[:].bitcast(F32))
                    nc.sync.dma_start(out=v_d.ap(), in_=v_sb[:])
                    nc.sync.dma_start(out=c_d.ap(), in_=corrS[:].bitcast(F32))

              nc.sync.dma_start(out=res_sb[:], in_=resid.ap())
              with tc.tile_pool(name="eps", bufs=3, space="PSUM") as eps:
                et0 = eps.tile([65, MTA], F32, tag="et")
                for mt in range(1, M // MTA):
                    et = eps.tile([65, MTA], F32, tag="et")
                    for kg in range(NKG):
                        idx = (mt - 1) * NKG + kg
                        if idx % 2 == 0 and idx // 2 < NKG:
                            pv_group(et0, idx // 2, exs0[idx // 2])
                        exs = st_exp(mt, kg)
                        pv_group(et, kg, exs)
                        if idx == 14:
                            normalize(0, et0)
                    normalize(mt, et)

    nc.compile()
    return nc


def _prep_core_inputs(inputs, core):
    A1_B = np.asarray(inputs["A1_B"], np.float32)
    A1_C = np.asarray(inputs["A1_C"], np.float32)
    w1 = np.asarray(inputs["w1"], np.float32)
    b1 = np.asarray(inputs["b1"], np.float32)
    w2 = np.asarray(inputs["w2"], np.float32)
    b2 = np.asarray(inputs["b2"], np.float32)
    w3 = np.asarray(inputs["w3"], np.float32)
    b3 = np.asarray(inputs["b3"], np.float32)
    b = core // 2
    half = core % 2
    h0 = half * 32

    xc = np.zeros((128, XC_LEN), np.float32)
    flat_c = np.zeros((C, H + 2, W), np.float32)
    flat_c[:, 1:H + 1, :] = A1_C[b]
    xc[0:C, 1:1 + (H + 2) * W] = flat_c.reshape(C, -1)
    xc[C:128, 0:XC_LEN - 64] = xc[0:C, 64:XC_LEN]

    xb = np.zeros((128, XB_LEN), np.float32)
    flat_b = np.zeros((C, 34, W), np.float32)
    glo = h0 - 1
    src_lo = max(glo, 0)
    src_hi = min(h0 + 33, H)
    flat_b[:, src_lo - glo: src_hi - glo, :] = A1_B[b][:, src_lo:src_hi, :]
    xb[0:C, 1:1 + 34 * W] = flat_b.reshape(C, -1)
    xb[C:128, 0:XB_LEN - 64] = xb[0:C, 64:XB_LEN]

    # Q weights, compact: tap t=dx holds (dy0 rows 0:64, dy1 rows 64:128),
    # taps 3+dx hold dy2 on rows 0:64.
    w1cm = np.zeros((128, 6 * 32), np.float32)
    w23t = np.zeros((128, 6 * 128), np.float32)
    for dx in range(3):
        w1cm[0:C, dx * 32:(dx + 1) * 32] = w1[:, :, 0, dx].T
        w1cm[C:128, dx * 32:(dx + 1) * 32] = w1[:, :, 1, dx].T
        w1cm[0:C, (3 + dx) * 32:(4 + dx) * 32] = w1[:, :, 2, dx].T
        w23t[0:C, dx * 128: dx * 128 + CH] = w2[:, :, 0, dx].T
        w23t[C:128, dx * 128: dx * 128 + CH] = w2[:, :, 1, dx].T
        w23t[0:C, dx * 128 + 64: dx * 128 + 128] = w3[:, :, 0, dx].T
        w23t[C:128, dx * 128 + 64: dx * 128 + 128] = w3[:, :, 1, dx].T
        w23t[0:C, (3 + dx) * 128: (3 + dx) * 128 + CH] = w2[:, :, 2, dx].T
        w23t[0:C, (3 + dx) * 128 + 64: (3 + dx) * 128 + 128] = w3[:, :, 2, dx].T
    wc1c = np.zeros((C, 6 * 32), np.float32)
    wcK = np.zeros((C, 6 * 32), np.float32)
    wcV = np.zeros((C, 6 * 64), np.float32)
    for dy in range(3):
        for side, dx in ((0, 0), (1, 2)):
            i = 2 * dy + side
            wc1c[:, i * 32:(i + 1) * 32] = -w1[:, :, dy, dx].T
            wcK[:, i * 32:(i + 1) * 32] = -w2[:, :, dy, dx].T
            wcV[:, i * 64:(i + 1) * 64] = -w3[:, :, dy, dx].T

    resid = np.ascontiguousarray(A1_C[b][:, h0:h0 + 32, :].reshape(C, M))
    xcr = _r32r(xc)
    xbr = _r32r(xb)

    def bf16(x):
        import ml_dtypes
        return np.ascontiguousarray(np.asarray(x, np.float32).astype(ml_dtypes.bfloat16))

    wcme = np.zeros((C, 970), np.float32)
    wcme[:, 0:192] = wc1c
    wcme[:, 192:384] = wcK
    wcme[:, 384:768] = wcV
    wcme[:, 768:834] = xcr[0:C, (np.arange(66)) * 64]
    wcme[:, 834:901] = xcr[0:C, (np.arange(67)) * 64 + 1]
    wcme[:, 901:935] = xbr[0:C, (np.arange(34)) * 64]
    wcme[:, 935:970] = xbr[0:C, (np.arange(35)) * 64 + 1]

    bia = np.zeros((128, 2), np.float32)
    bia[0:CH, 0] = b2
    bia[64:128, 0] = b3
    bia[0:CH, 1] = b1

    return {
        "xc": xcr,
        "xb": xbr,
        "w23t": _r32r(w23t),
        "w1c": _r32r(w1cm),
        "wcme": _r32r(wcme),
        "bia": bia,
        "resid": resid,
    }


def _run(inputs, trace=False, dbg=False):
    key = ("nc", dbg)
    if key not in _cache:
        _cache[key] = _build(dbg)
    nc = _cache[key]
    in_maps = [_prep_core_inputs(inputs, i) for i in range(NCORES)]
    res = bass_utils.run_bass_kernel_spmd(
        nc, in_maps, core_ids=list(range(NCORES)), trace=trace)
    out = np.empty((B, C, H, W), np.float32)
    for i in range(NCORES):
        b, half = i // 2, i % 2
        out[b, :, half * 32:half * 32 + 32, :] = res.results[i]["out"].reshape(C, 32, W)
    return out, res


def kernel(**inputs):
    out, _ = _run(inputs, trace=False)
    return out
